# revision 1
# baseline (speedup 1.0000x reference)
"""VP-SDE Euler-Maruyama forward diffusion on 8 Trainium2 NeuronCores.

Recurrence (per element, 100 steps):
    x_t = a_t * x_{t-1} + b_t * n_t
      a_t = 1 - 0.5 * beta_t * dt
      b_t = sqrt(beta_t * dt)
      beta_t = BETA0 + (t/S) * (BETA1 - BETA0)

Sharding: data-parallel over the batch dim (64 -> 8 per core). Each core
streams its noise shard from HBM, runs the elementwise recurrence on
ACT (noise pre-scale) + DVE (fused state update), and streams the
trajectory back out. Memory-bound: ~101 MiB of HBM traffic per core.
"""

import os

import numpy as np

import concourse.bass as bass
import concourse.mybir as mybir
from concourse.bass_utils import run_bass_kernel_spmd
from concourse.tile import TileContext

S = 100          # diffusion steps
N, L, D = 64, 256, 64
NCORES = 8
NB = N // NCORES           # batch per core
P = 128                    # SBUF partitions
F = NB * L * D // P        # free dim per step per core (1024)
K = 4                      # steps per DMA block (2 MiB transfers)

BETA0, BETA1 = 0.1, 20.0
DT = 1.0 / S

F32 = mybir.dt.float32

LAST_EXEC_NS = None


def _coeffs():
    """Per-step coefficients in rescaled space.

    x_t = a_t * x_{t-1} + b_t * n_t  is tracked as  y_t = y_{t-1} + s_t * n_t
    with x_t = gamma_t * y_t, gamma_t = prod(a_0..a_t), s_t = b_t / gamma_t.
    Keeps the serial chain a plain tensor_tensor add on one engine.
    """
    gammas, scales = [], []
    g = np.float64(1.0)
    for t in range(S):
        beta = np.float64(BETA0) + (t / S) * (BETA1 - BETA0)
        a = 1.0 - 0.5 * beta * DT
        b = np.sqrt(beta * DT)
        g = g * a
        gammas.append(float(g))
        scales.append(float(b / g))
    return gammas, scales


def _legalize_waits(nc, max_waits=1):
    """Split multi-sem waits into standalone EventSemaphore instructions.

    TRN2 TPB instruction encodings carry a single sem-wait slot; walrus
    rejects instructions with more ("Too many sync wait commands"). Tile
    emits up to 3 waits per instruction, so peel the excess onto
    same-engine EventSemaphore instructions placed immediately before —
    engine-queue program order makes this exactly equivalent.
    """
    split_types = tuple(
        t
        for t in (
            getattr(mybir, n, None)
            for n in (
                "InstTensorTensor",
                "InstActivation",
                "InstDMACopy",
                "InstTensorScalarPtr",
                "InstMemset",
                "InstTensorCopy",
                "InstTensorReduce",
                "InstCopy",
                "InstDrain",
            )
        )
        if t is not None
    )
    n = 0
    for fn in nc.m.functions:
        for blk in fn.blocks:
            out = []
            for inst in blk.instructions:
                si = inst.sync_info
                if (
                    si is not None
                    and si.on_wait
                    and len(si.on_wait) > max_waits
                    and isinstance(inst, split_types)
                ):
                    for w in si.on_wait[:-max_waits]:
                        n += 1
                        es = mybir.InstEventSemaphore(
                            name=f"legalize-wait-{n}", ins=[], outs=[]
                        )
                        es.name = f"legalize-wait-{n}"
                        es.engine = inst.engine
                        es.sync_info = mybir.SyncInfo(on_wait=[w], on_update=[])
                        nc.register_instruction(es)
                        out.append(es)
                    inst.sync_info = mybir.SyncInfo(
                        on_wait=list(si.on_wait[-max_waits:]),
                        on_update=list(si.on_update or []),
                    )
                out.append(inst)
            blk.instructions = out


def _build():
    # Partition-major DRAM layout: noise [P, S, F], out [P, S+1, F] so every
    # DMA moves one contiguous K*F*4-byte segment per partition.
    nc = bass.Bass()
    x = nc.declare_dram_parameter("x", [P, F], F32, isOutput=False)
    noise = nc.declare_dram_parameter("noise", [P, S, F], F32, isOutput=False)
    out = nc.declare_dram_parameter("out", [P, S + 1, F], F32, isOutput=True)
    G, SC = _coeffs()

    with TileContext(nc) as tc:
        with (
            tc.tile_pool(name="ypool", bufs=3) as ypool,
            tc.tile_pool(name="npool", bufs=3) as npool,
            tc.tile_pool(name="opool", bufs=3) as opool,
        ):
            yprev = ypool.tile([P, F], F32)
            nc.sync.dma_start(out=yprev[:], in_=x[:])
            nc.scalar.dma_start(out=out[:, 0, :], in_=yprev[:])
            for tb in range(0, S, K):
                ntile = npool.tile([P, K * F], F32)
                nc.sync.dma_start(
                    out=ntile[:],
                    in_=noise[:, tb : tb + K, :].rearrange("p s f -> p (s f)"),
                )
                otile = opool.tile([P, K * F], F32)
                for s in range(K):
                    t = tb + s
                    nslc = ntile[:, s * F : (s + 1) * F]
                    nc.scalar.mul(nslc, nslc, SC[t])
                    ycur = ypool.tile([P, F], F32, tag="ycur")
                    nc.vector.tensor_add(ycur[:], yprev[:], nslc)
                    nc.scalar.mul(otile[:, s * F : (s + 1) * F], ycur[:], G[t])
                    yprev = ycur
                nc.scalar.dma_start(
                    out=out[:, tb + 1 : tb + K + 1, :].rearrange("p s f -> p (s f)"),
                    in_=otile[:],
                )
    _legalize_waits(nc)
    return nc


_NC = None


def _install_trace_hook():
    """Register the axon NTFF profile hook (test-only; KERNEL_TRACE=1).

    The image's antenv package lacks axon_hooks, so run_bass_kernel_spmd's
    trace path degrades. Replicate the boot shim: drive NRT profiling via
    ctypes into libaxon_pjrt.so and seed sys.modules so bass_utils finds it.
    """
    import contextlib
    import ctypes
    import sys
    import types

    if "antenv.axon_hooks" in sys.modules:
        return
    so_path = "/opt/axon/libaxon_pjrt.so"
    lib = ctypes.CDLL(so_path)
    if not hasattr(lib, "axon_start_nrt_profile"):
        return
    lib.axon_start_nrt_profile.argtypes = [
        ctypes.POINTER(ctypes.c_int64),
        ctypes.c_size_t,
    ]
    lib.axon_start_nrt_profile.restype = ctypes.c_int64
    lib.axon_stop_nrt_profile.argtypes = [ctypes.c_char_p]
    lib.axon_stop_nrt_profile.restype = ctypes.c_int64

    @contextlib.contextmanager
    def _hook(output_dir, device_ids):
        import jax

        jax.devices()
        if device_ids:
            ids = (ctypes.c_int64 * len(device_ids))(*device_ids)
            rc = lib.axon_start_nrt_profile(ids, len(device_ids))
        else:
            rc = lib.axon_start_nrt_profile(None, 0)
        if rc != 0:
            raise RuntimeError(f"axon_start_nrt_profile rc={rc}")
        try:
            yield
        finally:
            n = lib.axon_stop_nrt_profile(str(output_dir).encode())
            print(f"profile: {n} file(s) written to {output_dir}", file=sys.stderr)

    mod = types.ModuleType("antenv.axon_hooks")
    mod.get_axon_ntff_profile_hook = lambda: _hook
    mod.set_axon_ntff_profile_hook = lambda h: None
    sys.modules["antenv.axon_hooks"] = mod

    # The trace path uploads NEFF artifacts to a remote bucket; no-op it.
    import concourse.bass_utils as _bu

    _bu.upload_artifacts = lambda tmpdir: tmpdir


def kernel(x: np.ndarray, noise: np.ndarray) -> np.ndarray:
    global _NC, LAST_EXEC_NS
    if _NC is None:
        _NC = _build()

    in_maps = []
    for c in range(NCORES):
        xs = np.ascontiguousarray(x[c * NB : (c + 1) * NB]).reshape(P, F)
        ns = np.ascontiguousarray(
            noise[:, c * NB : (c + 1) * NB]
            .reshape(S, P, F)
            .transpose(1, 0, 2)
        )
        in_maps.append({"x": xs, "noise": ns})

    trace = bool(os.environ.get("KERNEL_TRACE"))
    if trace:
        _install_trace_hook()
    res = run_bass_kernel_spmd(_NC, in_maps, list(range(NCORES)), trace=trace)
    LAST_EXEC_NS = res.exec_time_ns

    outs = [
        res.results[c]["out"]
        .transpose(1, 0, 2)
        .reshape(S + 1, NB, L, D)
        for c in range(NCORES)
    ]
    return np.concatenate(outs, axis=1)

